# revision 21
# baseline (speedup 1.0000x reference)
"""Trainium2 Bass kernel for nn_BuildModel_3796751089773.

RAIM-attention + LSTMCell scan over T=256 steps, B=1024, F=128, H=256, W=3,
followed by sum-over-time prediction head -> [B, 1].

Strategy (8 cores, data-parallel over batch, B_local = 128 = SBUF partitions):
  - Normal layout [batch_partitions, feature_free] for attention softmax and
    all elementwise work (per-partition scalars make softmax/z cheap).
  - gates = z @ W_ih^T + h @ W_hh^T computed with activations-transposed as
    PE stationary (hT/zT via PE transposes), weights streaming as rhs.
  - Output head sum_t(h_t) @ w_pred^T accumulated in a persistent PSUM bank
    by riding tiny N=1 matmuls on the already-loaded hT stationaries.
  - sigmoid(x) = 0.5*(1+tanh(x/2)) so the only ACT functions used are
    tanh/exp/copy -> one ACT table set ("exp_and_others"), loaded once.
  - h,c state kept doubled (H=2h, C=2c) so the LSTM update is exactly three
    fused scalar_tensor_tensor ops; the 0.5 factors fold into weights.
  - x-dependent parts of alpha/beta preacts fold into the SAME PSUM
    accumulation via 3 static rhs matrices; all three window contributions
    for step t are gathered at step t (from x_t, x_t-1, x_t-2) so only two
    ab PSUM tiles are ever alive, freeing a PSUM bank.
  - v2 latency optimizations (the scan is loop-carried-latency bound):
    * PE queue ordered so the z-critical path closes first: x-part matmuls,
      H transposes, both ab hT matmuls (stop) BEFORE the gate matmuls; two
      keep-warm transposes anchored on mid-ladder tensors write to y-PSUM
      scratch columns so the PE never idles >3.4us (HAM clock-gate stays
      warm = matmuls at 2.4 GHz).
    * trH0 and trH1 land in SEPARATE PSUM tiles so the hT0 copy's
      (whole-tile-tracked) dependency excludes the second transpose.
    * bf16 matmul path: weights, hT, zT, xT, identity and the LSTM
      pointwise tensors are bf16 (fp32r streams only 1 col per 1.2GHz cycle;
      bf16 streams at 2.4GHz -> every matmul halves). PSUM accumulation,
      softmax, u/z math stay fp32.
"""

import os
import sys

import numpy as np
import ml_dtypes

for _p in ("/opt/trn_rl_repo",):
    if _p not in sys.path:
        sys.path.insert(0, _p)

import concourse.bass as bass
import concourse.bacc as bacc
import concourse.tile as tile
from concourse import mybir
from concourse.bass_utils import run_bass_kernel_spmd
from concourse.masks import make_identity
from concourse.dve_ops import (
    OPS as _DVE_OPS, CUSTOM_DVE_SPECS as _DVE_SPECS,
    _SUB_OPCODE_FOR_NAME as _DVE_ROWS, _CUSTOM_DVE_ROW_BASE as _DVE_ROW_BASE,
    DveOp as _DveOp,
)
from concourse.dve_spec import Spec as _Spec, Src0 as _Src0, Src1 as _Src1, \
    C0 as _C0, C1 as _C1, lower as _dve_lower
from concourse.dve_uop import DveOpSpec as _DveOpSpec


def _register_u2_op():
    """out = in0*s0 + in1*s1 with two per-partition scalars (one DVE inst)."""
    name = "U2_MULADD_ANT"
    if name in _DVE_ROWS:
        return next(o for o in _DVE_OPS if o.name == name)
    spec = _Spec(
        body=_Src0 * _C0 + _Src1 * _C1,
        reference=lambda in0, in1, s0, s1, imm2:
            in0.astype(np.float32) * s0 + in1.astype(np.float32) * s1,
    )
    row = _DVE_ROW_BASE + len(_DVE_OPS)
    _DVE_ROWS[name] = row
    shas = {}
    for ver in ("v3", "v4"):
        try:
            uops = _dve_lower(spec, ver=ver)
            shas[ver] = _DveOpSpec(name=name, opcode=row, uops=uops,
                                   rd1_en=True).sha(ver)
        except Exception:
            pass
    op = _DveOp(name, spec, subdim=False, uops_sha=shas)
    _DVE_OPS.append(op)
    _DVE_SPECS[name] = spec
    return op


U2_OP = _register_u2_op()

B, T, F, W, H, L = 1024, 256, 128, 3, 256, 1
NCORES = 8
BL = B // NCORES  # 128
AB = 131   # used alpha(3)+beta(128) preact cols
ABP = 256  # PSUM/rhs width: float32r matmul needs N>=256 for full rate
AF = mybir.ActivationFunctionType
ALU = mybir.AluOpType
DT = mybir.dt
F32 = DT.float32
F32R = DT.float32r
BF16 = DT.bfloat16

_CACHE = {}


def build_kernel(gate_bias_nonzero, ab_bias_nonzero):
    nc = bacc.Bacc("TRN2")

    xn_d = nc.dram_tensor("xn", [T, BL, F], F32, kind="ExternalInput")
    xt_d = nc.dram_tensor("xt", [T, F, BL], BF16, kind="ExternalInput")
    wg_d = nc.dram_tensor("wg", [3, 128, 1024], BF16, kind="ExternalInput")
    wab_d = nc.dram_tensor("wab", [2, 128, ABP], BF16, kind="ExternalInput")
    wr_d = nc.dram_tensor("wr", [3, 128, ABP], BF16, kind="ExternalInput")
    wp_d = nc.dram_tensor("wp", [2, 128, 8], BF16, kind="ExternalInput")
    bg_d = nc.dram_tensor("bg", [1, 1024], BF16, kind="ExternalInput")
    bab_d = nc.dram_tensor("bab", [1, ABP], BF16, kind="ExternalInput")
    y_d = nc.dram_tensor("y", [BL, L], F32, kind="ExternalOutput")

    from contextlib import ExitStack

    with tile.TileContext(nc) as tc, ExitStack() as ctx:
        singles = ctx.enter_context(tc.tile_pool(name="singles", bufs=1))
        xn_pool = ctx.enter_context(tc.tile_pool(name="xn", bufs=3))
        xt_pool = ctx.enter_context(tc.tile_pool(name="xt", bufs=3))
        work = ctx.enter_context(tc.tile_pool(name="work", bufs=2))
        ab_pool = ctx.enter_context(tc.tile_pool(name="abps", bufs=2, space="PSUM"))
        g_pool = ctx.enter_context(tc.tile_pool(name="gps", bufs=1, space="PSUM"))
        tr_pool = ctx.enter_context(tc.tile_pool(name="trps", bufs=2, space="PSUM"))
        trb_pool = ctx.enter_context(tc.tile_pool(name="trbps", bufs=1, space="PSUM"))
        y_pool = ctx.enter_context(tc.tile_pool(name="yps", bufs=1, space="PSUM"))

        # ---- one-time loads -------------------------------------------------
        wg_s = singles.tile([128, 3, 1024], BF16)
        wab_s = singles.tile([128, 2, ABP], BF16)
        wr_s = singles.tile([128, 3, ABP], BF16)
        wp_s = singles.tile([128, 2, 8], BF16)
        for k in range(3):
            nc.sync.dma_start(out=wg_s[:, k, :], in_=wg_d[k])
            nc.sync.dma_start(out=wr_s[:, k, :], in_=wr_d[k])
        for k in range(2):
            nc.sync.dma_start(out=wab_s[:, k, :], in_=wab_d[k])
            nc.sync.dma_start(out=wp_s[:, k, :], in_=wp_d[k])
        ident = singles.tile([128, 128], BF16)
        make_identity(nc, ident)
        ident32 = singles.tile([128, 128], F32)
        make_identity(nc, ident32)
        ones_row = None
        bg_s = bab_s = None
        if gate_bias_nonzero or ab_bias_nonzero:
            ones_row = singles.tile([1, 128], BF16)
            nc.vector.memset(ones_row, 1.0)
        if gate_bias_nonzero:
            bg_s = singles.tile([1, 1024], BF16)
            nc.sync.dma_start(out=bg_s, in_=bg_d[:])
        if ab_bias_nonzero:
            bab_s = singles.tile([1, ABP], BF16)
            nc.sync.dma_start(out=bab_s, in_=bab_d[:])

        # ---- persistent state ----------------------------------------------
        H_s = singles.tile([128, 256], BF16)  # doubled hidden state 2*h
        C_s = singles.tile([128, 256], BF16)  # doubled cell state 2*c
        nc.gpsimd.memset(H_s, 0.0)
        nc.gpsimd.memset(C_s, 0.0)

        y_ps = y_pool.tile([128, 136], F32)  # [:, 0:8] = y accum, [:, 8:136] = keep-warm scratch

        ab_tiles = {}
        xn_tiles = {}


        GB = 4  # steps per DMA group
        xn_groups = {}
        xt_groups = {}
        for t in range(T):
            # -- DMA x slices in (batched, 4 steps per transfer) ---------------
            if t % GB == 0:
                gi = t // GB
                gxn = xn_pool.tile([128, GB, 128], F32, tag="xn",
                                   name=f"xng{gi}")
                nc.sync.dma_start(out=gxn,
                                  in_=xn_d[t:t + GB].rearrange("k p f -> p k f"))
                xn_groups[gi] = gxn
                gxt = xt_pool.tile([128, GB, 128], BF16, tag="xt",
                                   name=f"xtg{gi}")
                nc.sync.dma_start(out=gxt,
                                  in_=xt_d[t:t + GB].rearrange("k p f -> p k f"))
                xt_groups[gi] = gxt
            xn_tiles[t] = xn_groups[t // GB][:, t % GB, :]
            xt_tiles = {s: xt_groups[s // GB][:, s % GB, :]
                        for s in range(max(0, t - 2), t + 1)}
            xt_t = xt_tiles[t]

            # -- all three x-window contributions for THIS step's ab tile ------
            ab_t = ab_pool.tile([128, ABP], F32, tag="ab", name=f"ab{t}")
            ab_tiles[t] = ab_t
            if ab_bias_nonzero:
                nc.tensor.matmul(ab_t, ones_row, bab_s, start=True, stop=False)
            first = not ab_bias_nonzero
            if t >= 2:
                nc.tensor.matmul(ab_t, xt_tiles[t - 2], wr_s[:, 0, :],
                                 start=first, stop=False)
                first = False
            if t >= 1:
                nc.tensor.matmul(ab_t, xt_tiles[t - 1], wr_s[:, 1, :],
                                 start=first, stop=False)
                first = False
            nc.tensor.matmul(ab_t, xt_t, wr_s[:, 2, :],
                             start=first, stop=False)

            # -- transpose H (state from step t-1); close ab ASAP --------------
            tr_t = tr_pool.tile([128, 256], BF16, tag="tr")
            trb_t = trb_pool.tile([128, 128], BF16, tag="trb")
            hT01 = work.tile([128, 256], BF16, tag="hT01")
            hT0 = hT01[:, 0:128]
            hT1 = hT01[:, 128:256]
            nc.tensor.transpose(tr_t[:, 0:128], H_s[:, 0:128], ident)
            nc.tensor.transpose(trb_t, H_s[:, 128:256], ident)
            nc.vector.tensor_copy(out=hT0, in_=tr_t[:, 0:128])
            nc.vector.tensor_copy(out=hT1, in_=trb_t)
            nc.tensor.matmul(ab_tiles[t], hT0, wab_s[:, 0, :],
                             start=False, stop=False)
            nc.tensor.matmul(ab_tiles[t], hT1, wab_s[:, 1, :],
                             start=False, stop=True)

            # -- gates-h matmuls (reuse hT1 stationary first, then hT0) --------
            g0_ps = g_pool.tile([128, 512], F32, tag="g0")
            g1_ps = g_pool.tile([128, 512], F32, tag="g1")
            nc.tensor.matmul(g0_ps, hT1, wg_s[:, 2, 0:512],
                             start=not gate_bias_nonzero, stop=False)
            nc.tensor.matmul(g1_ps, hT1, wg_s[:, 2, 512:1024],
                             start=not gate_bias_nonzero, stop=False)
            nc.tensor.matmul(y_ps[:, 0:8], hT1, wp_s[:, 1, :],
                             start=(t == 0), stop=False)
            nc.tensor.matmul(g0_ps, hT0, wg_s[:, 1, 0:512],
                             start=False, stop=False)
            nc.tensor.matmul(g1_ps, hT0, wg_s[:, 1, 512:1024],
                             start=False, stop=False)
            nc.tensor.matmul(y_ps[:, 0:8], hT0, wp_s[:, 0, :],
                             start=False, stop=False)

            # -- attention softmax path ----------------------------------------
            t_ab = work.tile([128, AB], F32, tag="tab")
            nc.scalar.activation(out=t_ab, in_=ab_tiles[t][:, 0:AB], func=AF.Tanh)
            e_a = work.tile([128, 3], F32, tag="ea")
            e_b = work.tile([128, 128], F32, tag="eb")
            s_a = work.tile([128, 1], F32, tag="sa")
            s_b = work.tile([128, 1], F32, tag="sb")
            nc.scalar.activation(out=e_a, in_=t_ab[:, 0:3], func=AF.Exp,
                                 accum_out=s_a)
            nc.scalar.activation(out=e_b, in_=t_ab[:, 3:AB], func=AF.Exp,
                                 accum_out=s_b)

            # u = sum_w e_alpha[w] * x_{t-2+w}
            u = work.tile([128, 128], F32, tag="u")
            if t == 0:
                nc.vector.tensor_scalar_mul(u, xn_tiles[0], e_a[:, 2:3])
            elif t == 1:
                nc.vector._custom_dve(
                    U2_OP, out=u, in0=xn_tiles[0], in1=xn_tiles[1],
                    s0=e_a[:, 1:2], s1=e_a[:, 2:3])
            else:
                u01 = work.tile([128, 128], F32, tag="u01")
                nc.vector._custom_dve(
                    U2_OP, out=u01, in0=xn_tiles[t - 2], in1=xn_tiles[t - 1],
                    s0=e_a[:, 0:1], s1=e_a[:, 1:2])
                nc.vector.scalar_tensor_tensor(
                    out=u, in0=xn_tiles[t], scalar=e_a[:, 2:3], in1=u01,
                    op0=ALU.mult, op1=ALU.add)
            s_ab = work.tile([128, 1], F32, tag="sab")
            nc.vector.tensor_mul(s_ab, s_a, s_b)
            r_ab = work.tile([128, 1], F32, tag="rab")
            nc.vector.reciprocal(r_ab, s_ab)
            # z = e_beta * u * r  (normalized attention output)
            z = work.tile([128, 128], BF16, tag="z")
            nc.vector.scalar_tensor_tensor(
                out=z, in0=u, scalar=r_ab, in1=e_b,
                op0=ALU.mult, op1=ALU.mult)

            # -- zT and gates-z -------------------------------------------------
            nc.tensor.transpose(tr_t[:, 128:256], z, ident)
            zT = work.tile([128, 128], BF16, tag="zT")
            nc.scalar.copy(out=zT, in_=tr_t[:, 128:256])
            if gate_bias_nonzero:
                nc.tensor.matmul(g0_ps, ones_row, bg_s[:, 0:512],
                                 start=False, stop=False)
                nc.tensor.matmul(g1_ps, ones_row,
                                 bg_s[:, 512:1024], start=False, stop=False)
            nc.tensor.matmul(g0_ps, zT, wg_s[:, 0, 0:512],
                             start=False, stop=True)
            nc.tensor.matmul(g1_ps, zT, wg_s[:, 0, 512:1024],
                             start=False, stop=True)


            # -- gate activations: cols [f(256) i(256) o(256) g(256)] ----------
            tg4 = work.tile([128, 1024], BF16, tag="tg4")
            nc.scalar.activation(out=tg4[:, 0:512], in_=g0_ps,
                                 func=AF.Tanh, scale=0.5)

            # -- LSTM state update (doubled state) ------------------------------
            # A = (1+tanh(f/2)) * C   (= 4*sig(f)*c)
            A_t = work.tile([128, 256], F32, tag="A")
            nc.vector.scalar_tensor_tensor(
                out=A_t, in0=tg4[:, 0:256], scalar=1.0, in1=C_s,
                op0=ALU.add, op1=ALU.mult)
            nc.scalar.activation(out=tg4[:, 768:1024], in_=g1_ps[:, 256:512],
                                 func=AF.Tanh)
            # Q = (1+tanh(i/2)) * tanh(g)   (= 2*sig(i)*tanh(g))
            Q_t = work.tile([128, 256], BF16, tag="Q")
            nc.vector.scalar_tensor_tensor(
                out=Q_t, in0=tg4[:, 256:512], scalar=1.0, in1=tg4[:, 768:1024],
                op0=ALU.add, op1=ALU.mult)
            nc.scalar.activation(out=tg4[:, 512:768], in_=g1_ps[:, 0:256],
                                 func=AF.Tanh, scale=0.5)
            # C_new = 0.5*A + Q   (= 2*c_new)
            nc.vector.scalar_tensor_tensor(
                out=C_s, in0=A_t, scalar=0.5, in1=Q_t,
                op0=ALU.mult, op1=ALU.add)
            # tanh(c_new) = tanh(0.5*C)
            t_c = work.tile([128, 256], F32, tag="tc")
            nc.scalar.activation(out=t_c, in_=C_s, func=AF.Tanh, scale=0.5)
            # H_new = (1+tanh(o/2)) * tanh(c_new)   (= 2*h_new), in halves so
            # the H0 transpose/copy/ab-matmul overlap the H1 half of the tail.
            nc.vector.scalar_tensor_tensor(
                out=H_s[:, 0:128], in0=tg4[:, 512:640], scalar=1.0,
                in1=t_c[:, 0:128], op0=ALU.add, op1=ALU.mult)
            nc.vector.scalar_tensor_tensor(
                out=H_s[:, 128:256], in0=tg4[:, 640:768], scalar=1.0,
                in1=t_c[:, 128:256], op0=ALU.add, op1=ALU.mult)

        # ---- final h contribution to y + writeback --------------------------
        tr_f = tr_pool.tile([128, 256], BF16, tag="tr")
        trb_f = trb_pool.tile([128, 128], BF16, tag="trb")
        nc.tensor.transpose(tr_f[:, 0:128], H_s[:, 0:128], ident)
        nc.tensor.transpose(trb_f, H_s[:, 128:256], ident)
        hT01f = work.tile([128, 256], BF16, tag="hT01")
        nc.vector.tensor_copy(out=hT01f[:, 0:128], in_=tr_f[:, 0:128])
        nc.vector.tensor_copy(out=hT01f[:, 128:256], in_=trb_f)
        hT0f = hT01f[:, 0:128]
        hT1f = hT01f[:, 128:256]
        nc.tensor.matmul(y_ps[:, 0:8], hT0f, wp_s[:, 0, :], start=False, stop=False)
        nc.tensor.matmul(y_ps[:, 0:8], hT1f, wp_s[:, 1, :], start=False, stop=True)
        y_sb = work.tile([128, 1], F32, tag="ysb")
        nc.scalar.copy(out=y_sb, in_=y_ps[:, 0:1])
        nc.sync.dma_start(out=y_d[:], in_=y_sb)

    nc.finalize()
    return nc


def _prep_inputs(v, w_h_alpha, b_h_alpha, w_a_alpha, b_a_alpha,
                 w_h_beta, b_h_beta, w_a_beta, b_a_beta,
                 w_ih, b_ih, w_hh, b_hh, w_pred, b_pred):
    v = np.ascontiguousarray(np.asarray(v, dtype=np.float32))
    # gate row reorder: torch order (i,f,g,o) -> (f,i,o,g)
    idx = np.concatenate([np.arange(H, 2 * H), np.arange(0, H),
                          np.arange(3 * H, 4 * H), np.arange(2 * H, 3 * H)])
    wih_p = np.asarray(w_ih, np.float32)[idx]          # [1024, 128]
    whh_p = np.asarray(w_hh, np.float32)[idx]          # [1024, 256]
    bg = (np.asarray(b_ih, np.float32) + np.asarray(b_hh, np.float32))[idx]

    wg = np.zeros((3, 128, 1024), np.float32)
    wg[0] = wih_p.T
    wg[1] = 0.5 * whh_p.T[0:128]
    wg[2] = 0.5 * whh_p.T[128:256]

    wab = np.zeros((2, 128, ABP), np.float32)
    wha_t = np.asarray(w_h_alpha, np.float32).T        # [H, 3]
    whb_t = np.asarray(w_h_beta, np.float32).T         # [H, F]
    for k in range(2):
        wab[k, :, 0:3] = 0.5 * wha_t[128 * k:128 * (k + 1)]
        wab[k, :, 3:AB] = 0.5 * whb_t[128 * k:128 * (k + 1)]

    wr = np.zeros((3, 128, ABP), np.float32)
    waa = np.asarray(w_a_alpha, np.float32)[0]         # [F]
    wab_beta = np.asarray(w_a_beta, np.float32)[0]     # [W]
    eye = np.eye(128, dtype=np.float32)
    for d in range(3):
        wr[d, :, d] = waa
        wr[d, :, 3:AB] = wab_beta[d] * eye

    wp = np.zeros((2, 128, 8), np.float32)
    wp[:, :, 0] = (0.5 * np.asarray(w_pred, np.float32)[0]).reshape(2, 128)

    bab = np.zeros((1, ABP), np.float32)
    bab[0, 0:3] = np.asarray(b_h_alpha, np.float32) + np.asarray(b_a_alpha,
                                                                 np.float32)[0]
    bab[0, 3:AB] = np.asarray(b_h_beta, np.float32) + np.asarray(b_a_beta,
                                                                 np.float32)[0]

    gate_bias_nonzero = bool(np.any(bg != 0.0))
    ab_bias_nonzero = bool(np.any(bab != 0.0))

    b16 = ml_dtypes.bfloat16
    shared = {
        "wg": wg.astype(b16), "wab": wab.astype(b16), "wr": wr.astype(b16),
        "wp": wp.astype(b16),
        "bg": bg.reshape(1, 1024).astype(b16), "bab": bab.astype(b16),
    }
    in_maps = []
    vs = v.reshape(NCORES, BL, T, F)
    for c in range(NCORES):
        vc = vs[c]                                     # [BL, T, F]
        in_maps.append({
            "xn": np.ascontiguousarray(vc.transpose(1, 0, 2)),  # [T, BL, F]
            "xt": np.ascontiguousarray(vc.transpose(1, 2, 0)).astype(b16),
            **shared,
        })
    b_pred_total = float(T) * np.asarray(b_pred, np.float32)    # [L]
    return in_maps, gate_bias_nonzero, ab_bias_nonzero, b_pred_total


def _run(inputs, trace=False):
    in_maps, gb_nz, ab_nz, b_pred_total = _prep_inputs(**inputs)
    key = (gb_nz, ab_nz)
    if key not in _CACHE:
        _CACHE[key] = build_kernel(gb_nz, ab_nz)
    nc = _CACHE[key]
    res = run_bass_kernel_spmd(
        nc, in_maps, core_ids=list(range(NCORES)), trace=trace,
    )
    y = np.concatenate([res.results[c]["y"] for c in range(NCORES)], axis=0)
    y = y + b_pred_total[None, :]
    return np.asarray(y, dtype=np.float32), res


def kernel(**inputs):
    y, _ = _run(inputs, trace=False)
    return y
